# revision 1
# baseline (speedup 1.0000x reference)
"""Multi-head self-attention (B=2, S=2048, D=1024, H=16, causal) on 8 TRN2 NeuronCores.

Sharding: data parallel over batch (2) x tensor parallel over heads (4 groups of 4).
Core c handles batch c//4, heads 4*(c%4) .. 4*(c%4)+4; host sums 4 partials/batch.

Design (v4):
- Q/K projections: 3-term bf16 hi/lo (wh*xh + wh*xl + wl*xh) accumulated in fp32
  PSUM (~1e-5 relative q/k accuracy; fp32r is unusable here - its data path
  carries only ~12 mantissa bits end to end).
- Q/K stored as bf16 hi + bf16 lo pairs (hi = bf16(q), lo = q - hi); scores are
  3 accumulated bf16 matmuls qh*kh + qh*kl + ql*kh; causal mask added via a
  small bf16 matmul (-3e30 upper triangle) on the diagonal block.
- Softmax: row max via DVE tensor_reduce(negate) on PSUM score chunks; exp on
  Act (bias=-max) writes P fp16 to SBUF; P^T via DMA-engine transpose
  (SBUF->SBUF 16x128 xbar tiles) - no PE transposes, no PSUM->SBUF P copies;
  softmax denominator comes free from a ones-column appended to V; per-head
  1/sum is fused into the Act copy (Copy activation with AP scale) of attn@V.
- V projection: 1-term bf16, written as fp16 (with ones column) for attn@V.
- Output projection bf16 via DMA-transposed attn output; out DMA on HWDGE.
"""
import sys
for _p in ("/opt/trn_rl_repo", "/root/.axon_site/_ro/trn_rl_repo"):
    if _p not in sys.path:
        sys.path.append(_p)

import math
from contextlib import ExitStack

import numpy as np
import ml_dtypes

import concourse.bass as bass
import concourse.bacc as bacc
import concourse.tile as tile
import concourse.mybir as mybir
from concourse.bass_utils import run_bass_kernel_spmd

BF16 = mybir.dt.bfloat16
FP16 = mybir.dt.float16
F32 = mybir.dt.float32
SEQ = 2048
DM = 1024
DL = 256          # local head dims (4 heads x 64)
DH = 64
MC = 8            # 128-row chunks of the model dim
NQT = SEQ // 128  # 16 q tiles
NEG = -3.0e30     # causal mask addend
VW = 66           # v columns per head: 64 + ones col + pad

_CACHE = {}


def build_nc(pbufs=5, sbufs=6, mbufs=2, depth=None, pop_chunk=False, actq=False, SPLIT_QI=14, UPFRONT=1):
    nc = bacc.Bacc("TRN2", debug=False, num_devices=8)

    xh_d = nc.dram_tensor("xh", [MC, 128, SEQ], BF16, kind="ExternalInput")
    xl_d = nc.dram_tensor("xl", [MC, 128, SEQ], BF16, kind="ExternalInput")
    wqh_d = nc.dram_tensor("wqh", [MC, 128, DL], BF16, kind="ExternalInput")
    wql_d = nc.dram_tensor("wql", [MC, 128, DL], BF16, kind="ExternalInput")
    wkh_d = nc.dram_tensor("wkh", [MC, 128, DL], BF16, kind="ExternalInput")
    wkl_d = nc.dram_tensor("wkl", [MC, 128, DL], BF16, kind="ExternalInput")
    wvh_d = nc.dram_tensor("wvh", [MC, 128, DL], BF16, kind="ExternalInput")
    poT_d = nc.dram_tensor("poT", [2, 128, DM], BF16, kind="ExternalInput")
    mask_d = nc.dram_tensor("mask", [128, 128], BF16, kind="ExternalInput")
    ident_d = nc.dram_tensor("ident", [128, 128], BF16, kind="ExternalInput")
    out_d = nc.dram_tensor("out_part", [SEQ, DM], BF16, kind="ExternalOutput")

    X = mybir.AxisListType.X
    EXPF = mybir.ActivationFunctionType.Exp
    CPY = mybir.ActivationFunctionType.Copy

    with tile.TileContext(nc) as tc, ExitStack() as ctx:
        cst = ctx.enter_context(tc.tile_pool(name="cst", bufs=1))
        pp = ctx.enter_context(tc.tile_pool(name="pp", bufs=pbufs))
        ptp = ctx.enter_context(tc.tile_pool(name="ptp", bufs=pbufs))
        catp = ctx.enter_context(tc.tile_pool(name="catp", bufs=2))
        outp = ctx.enter_context(tc.tile_pool(name="outp", bufs=2))
        small = ctx.enter_context(tc.tile_pool(name="small", bufs=12))
        sp = ctx.enter_context(tc.tile_pool(name="sp", bufs=sbufs, space="PSUM"))
        mp = ctx.enter_context(tc.tile_pool(name="mp", bufs=mbufs, space="PSUM"))

        # ---- persistent SBUF loads ----
        mask_sb = cst.tile([128, 128], BF16, tag="mask")
        nc.sync.dma_start(out=mask_sb, in_=mask_d[:, :])
        ident_sb = cst.tile([128, 128], BF16, tag="ident")
        nc.sync.dma_start(out=ident_sb, in_=ident_d[:, :])
        poT_sb = cst.tile([128, 2, DM], BF16, tag="poT")
        for m in range(2):
            nc.sync.dma_start(out=poT_sb[:, m, :], in_=poT_d[m])
        xh_sb = cst.tile([128, MC, SEQ], BF16, tag="xh")
        xl_sb = cst.tile([128, MC, SEQ], BF16, tag="xl")
        wsb = {}
        for nm_ in ("wqh", "wql", "wkh", "wkl", "wvh"):
            wsb[nm_] = cst.tile([128, MC, DL], BF16, tag=nm_, name=nm_)
        wdr = dict(wqh=wqh_d, wql=wql_d, wkh=wkh_d, wkl=wkl_d, wvh=wvh_d)
        for m in range(MC):
            for nm_ in ("wqh", "wql", "wkh", "wkl", "wvh"):
                nc.sync.dma_start(out=wsb[nm_][:, m, :], in_=wdr[nm_][m])
            nc.sync.dma_start(out=xh_sb[:, m, :], in_=xh_d[m])
            nc.sync.dma_start(out=xl_sb[:, m, :], in_=xl_d[m])

        v_sb = cst.tile([128, NQT, 4, VW], FP16, tag="v")
        nc.gpsimd.memset(v_sb[:, :, :, 64:VW], 0.0)
        nc.gpsimd.memset(v_sb[:, :, :, 64:65], 1.0)

        # stacked bf16 hi/lo per head: Qstk=[ql(0:64); qh(64:128)],
        # Kstk=[kh(0:64); kl(64:128)] -> one 128-part matmul Qstk*Kstk gives
        # ql*kh + qh*kl (both cross terms merged); QH holds qh at base 0 so
        # the hi*hi term qh*kh pairs with Kstk's kh at matching partitions.
        Qstk = cst.tile([128, 4, SEQ], BF16, tag="Qstk")
        Kstk = cst.tile([128, 4, SEQ], BF16, tag="Kstk")
        QH = cst.tile([64, 4, SEQ], BF16, tag="QH")
        stgp = ctx.enter_context(tc.tile_pool(name="stgp", bufs=4))

        # ---- projections (3-term bf16 hi/lo) + hi/lo split of q/k ----
        # one chain = one (tensor, dc) pair for one 512-seq span
        PCH = ((wsb["wqh"], wsb["wql"], Qstk, False, 0),
               (wsb["wqh"], wsb["wql"], Qstk, False, 1),
               (wsb["wkh"], wsb["wkl"], Kstk, True, 0),
               (wsb["wkh"], wsb["wkl"], Kstk, True, 1))

        _mv = [0]

        def _mvq():
            _mv[0] += 1
            return nc.sync if _mv[0] % 2 else nc.gpsimd

        def split_and_stack(pr, j, span, q=None):
            wh_t, wl_t, STK, is_k, dc = PCH[j]
            sh = stgp.tile([128, 512], BF16, tag="sh", name="sh")
            nc.scalar.copy(out=sh, in_=pr)
            sl = stgp.tile([128, 512], BF16, tag="sl", name="sl")
            nc.vector.tensor_sub(sl, pr, sh)
            for half in range(2):
                hl = 2 * dc + half
                shh = sh[64 * half:64 * (half + 1), :]
                sll = sl[64 * half:64 * (half + 1), :]
                if is_k:
                    # Kstk=[kh(0:64); kl(64:128)]
                    if half == 0:
                        nc.vector.tensor_copy(out=Kstk[0:64, hl, span], in_=shh)
                        (q or _mvq()).dma_start(out=Kstk[64:128, hl, span], in_=sll)
                    else:
                        (q or _mvq()).dma_start(out=Kstk[0:64, hl, span], in_=shh)
                        nc.vector.tensor_copy(out=Kstk[64:128, hl, span], in_=sll)
                else:
                    # Qstk=[ql(0:64); qh(64:128)], QH=[qh] at base 0
                    if half == 0:
                        nc.scalar.copy(out=QH[0:64, hl, span], in_=shh)
                        (q or _mvq()).dma_start(out=Qstk[64:128, hl, span], in_=shh)
                        nc.vector.tensor_copy(out=Qstk[0:64, hl, span], in_=sll)
                    else:
                        (q or _mvq()).dma_start(out=QH[0:64, hl, span], in_=shh)
                        nc.scalar.copy(out=Qstk[64:128, hl, span], in_=shh)
                        (q or _mvq()).dma_start(out=Qstk[0:64, hl, span], in_=sll)

        def emit_proj_chain(qc, j):
            span = slice(512 * qc, 512 * (qc + 1))
            wh_t, wl_t, STK, is_k, dc = PCH[j]
            pr = mp.tile([128, 512], F32, tag="m", name="pr")
            n = 0
            for m in range(MC):
                for lt_, rt_ in ((wh_t, xh_sb), (wh_t, xl_sb), (wl_t, xh_sb)):
                    nc.tensor.matmul(
                        pr, lt_[:, m, 128 * dc:128 * (dc + 1)],
                        rt_[:, m, span], start=(n == 0), stop=(n == 3 * MC - 1))
                    n += 1
            split_and_stack(pr, j, span)

        def emit_proj_interleaved(qc):
            span = slice(512 * qc, 512 * (qc + 1))
            prs = []
            for j in range(4):
                prj = sp.tile([128, 512], F32, tag="s", name=f"pr{j}")
                prs.append(prj)
            for m in range(MC):
                for j in range(4):
                    wh_t, wl_t, STK, is_k, dc = PCH[j]
                    n0 = 3 * m
                    for i, (lt_, rt_) in enumerate(((wh_t, xh_sb), (wh_t, xl_sb),
                                                    (wl_t, xh_sb))):
                        nc.tensor.matmul(
                            prs[j], lt_[:, m, 128 * dc:128 * (dc + 1)],
                            rt_[:, m, span], start=(n0 + i == 0),
                            stop=(n0 + i == 3 * MC - 1))
            for j in range(4):
                split_and_stack(prs[j], j, span, q=nc.sync)

        def emit_v(qi):
            vps = mp.tile([128, 4, 64], F32, tag="m", name="vps")
            for m in range(MC):
                nc.tensor.matmul(vps, xh_sb[:, m, 128 * qi:128 * (qi + 1)],
                                 wsb["wvh"][:, m, :], start=(m == 0), stop=(m == MC - 1))
            nc.vector.tensor_copy(out=v_sb[:, qi, :, 0:64], in_=vps)

        # ---- attention: software-pipelined emission ----
        # Back-halves (attnV+scale) and outproj are emitted a couple of PE work
        # units after their dependencies' producers, so the in-order PE queue
        # never head-of-line blocks on the exp -> DMA-transpose chain.
        pending = []

        def pop1():
            if pending:
                pending.pop(0)()

        def push(fn):
            pending.append(fn)
            if depth is not None:
                while len(pending) > depth:
                    pending.pop(0)()

        state = {}
        for qi in range(NQT):
            if qi == 0:
                for qc0 in range(UPFRONT):
                    emit_proj_interleaved(qc0)
            elif qi <= 6:
                for ci in (2 * qi + 2, 2 * qi + 3):
                    if ci <= 15:
                        emit_proj_chain(ci // 4, ci % 4)
            emit_v(qi)
            nkt = qi + 1
            kend = 128 * nkt
            cat = catp.tile([128, DL], BF16, tag="cat", name="cat")
            for hp in range(2):
                for e in range(2):
                    hl = 2 * hp + e
                    qspan = slice(128 * qi, 128 * (qi + 1))
                    qh = QH[0:64, hl, qspan]
                    q2 = Qstk[:, hl, qspan]
                    s_tiles = []
                    nms = []
                    for base in range(0, kend, 512):
                        cw = min(512, kend - base)
                        S = sp.tile([128, 512], F32, tag="s", name="S")
                        s_tiles.append((S, base, cw))
                        gs = slice(base, base + cw)
                        has_diag = base <= 128 * qi < base + cw
                        nc.tensor.matmul(S[:, :cw], qh, Kstk[0:64, hl, gs],
                                         start=True, stop=False)
                        nc.tensor.matmul(S[:, :cw], q2, Kstk[:, hl, gs],
                                         start=False, stop=not has_diag)
                        if has_diag:
                            off = 128 * qi - base
                            nc.tensor.matmul(S[:, off:off + 128], ident_sb,
                                             mask_sb, start=False, stop=True)
                        nmc = small.tile([128, 1], F32, tag="nm", name="nmc")
                        nc.vector.tensor_reduce(out=nmc, in_=S[:, :cw], axis=X,
                                                op=mybir.AluOpType.max, negate=True)
                        nms.append(nmc)
                        if pop_chunk:
                            pop1()
                    lvl = list(nms)
                    while len(lvl) > 1:
                        nxt = []
                        for i in range(0, len(lvl) - 1, 2):
                            nm2 = small.tile([128, 1], F32, tag="nm", name="nm2")
                            nc.vector.tensor_tensor(out=nm2, in0=lvl[i], in1=lvl[i + 1],
                                                    op=mybir.AluOpType.min)
                            nxt.append(nm2)
                        if len(lvl) % 2:
                            nxt.append(lvl[-1])
                        lvl = nxt
                    nm = lvl[0]
                    p = pp.tile([128, SEQ], FP16, tag="p", name="p")
                    for S, base, cw in s_tiles:
                        nc.scalar.activation(out=p[:, base:base + cw], in_=S[:, :cw],
                                             func=EXPF, bias=nm, scale=1.0)
                    pt = ptp.tile([128, NQT, 128], FP16, tag="pt", name="pt")
                    if qi >= SPLIT_QI:
                        nc.sync.dma_start_transpose(out=pt[:, :8, :], in_=p[:, :1024])
                        nc.sync.dma_start_transpose(out=pt[:, 8:nkt, :],
                                                    in_=p[:, 1024:kend])
                    else:
                        nc.sync.dma_start_transpose(out=pt[:, :nkt, :], in_=p[:, :kend])

                    def back(qi=qi, hp=hp, e=e, hl=hl, pt=pt, nkt=nkt, cat=cat):
                        if e == 0:
                            state[(qi, hp)] = mp.tile([128, 2, VW], F32, tag="m",
                                                      name="av")
                        av = state[(qi, hp)]
                        for kt in range(nkt):
                            nc.tensor.matmul(av[:, e, :], pt[:, kt, :],
                                             v_sb[:, kt, hl, :],
                                             start=(kt == 0), stop=(kt == nkt - 1))
                        if e == 1:
                            inv = small.tile([128, 2], F32, tag="inv", name="inv")
                            nc.vector.reciprocal(out=inv, in_=av[:, :, 64])
                            for ee in range(2):
                                nc.scalar.activation(
                                    out=cat[:, 128 * hp + 64 * ee:128 * hp + 64 * (ee + 1)],
                                    in_=av[:, ee, 0:64], func=CPY,
                                    scale=inv[:, ee:ee + 1], bias=0.0)
                            del state[(qi, hp)]
                            if hp == 1:
                                acT = catp.tile([128, 2, 128], BF16, tag="acT",
                                                name="acT")
                                (nc.scalar if actq else nc.sync).dma_start_transpose(
                                    out=acT, in_=cat)
                                state[("acT", qi)] = acT

                    if depth is None:
                        pop1()
                    push(back)

            def outproj(qi=qi, cat=cat):
                acT = state.pop(("acT", qi))
                osb = outp.tile([128, DM], BF16, tag="osb", name="osb")
                for nci in range(2):
                    ops = mp.tile([128, 512], F32, tag="m", name="ops")
                    for mlc in range(2):
                        nc.tensor.matmul(ops, acT[:, mlc, :],
                                         poT_sb[:, mlc, 512 * nci:512 * (nci + 1)],
                                         start=(mlc == 0), stop=(mlc == 1))
                    if nci == 0:
                        nc.scalar.copy(out=osb[:, 512 * nci:512 * (nci + 1)], in_=ops)
                    else:
                        nc.vector.tensor_copy(out=osb[:, 512 * nci:512 * (nci + 1)], in_=ops)
                (nc.scalar if actq else nc.sync).dma_start(out=out_d[128 * qi:128 * (qi + 1), :], in_=osb)

            push(outproj)
            if depth is None:
                pop1()
        while pending:
            pending.pop(0)()

    nc.compile()
    return nc


def _bf16(a):
    return a.astype(ml_dtypes.bfloat16)


def _split(a):
    hi = _bf16(a)
    lo = _bf16(a - hi.astype(np.float32))
    return hi, lo


def _prep_inputs(x, p_q, p_k, p_v, p_o):
    """Build the 8 per-core input maps."""
    per_batch = []
    for b in range(2):
        xT = np.ascontiguousarray(x[b].T).astype(np.float32)  # [1024, 2048]
        xh, xl = _split(xT)
        per_batch.append((xh.reshape(MC, 128, SEQ), xl.reshape(MC, 128, SEQ)))

    mask = np.zeros((128, 128), np.float32)
    iu = np.triu_indices(128, 1)
    mask[iu] = NEG
    mask = _bf16(mask)
    ident = np.eye(128, dtype=ml_dtypes.bfloat16)

    per_group = []
    for g in range(4):
        rows = slice(DL * g, DL * (g + 1))
        wqT = np.ascontiguousarray((p_q[rows] / math.sqrt(DH)).T).astype(np.float32)
        wkT = np.ascontiguousarray(p_k[rows].T).astype(np.float32)
        wvT = np.ascontiguousarray(p_v[rows].T).astype(np.float32)
        poT = np.ascontiguousarray(p_o[:, rows].T).astype(ml_dtypes.bfloat16)
        wqh, wql = _split(wqT)
        wkh, wkl = _split(wkT)
        per_group.append(dict(
            wqh=wqh.reshape(MC, 128, DL), wql=wql.reshape(MC, 128, DL),
            wkh=wkh.reshape(MC, 128, DL), wkl=wkl.reshape(MC, 128, DL),
            wvh=_bf16(wvT).reshape(MC, 128, DL), poT=poT.reshape(2, 128, DM),
        ))

    in_maps = []
    for c in range(8):
        b, g = c // 4, c % 4
        m = dict(per_group[g])
        m["xh"], m["xl"] = per_batch[b]
        m["mask"] = mask
        m["ident"] = ident
        in_maps.append(m)
    return in_maps


def kernel(x, p_q, p_k, p_v, p_o):
    if "nc" not in _CACHE:
        _CACHE["nc"] = build_nc()
    nc = _CACHE["nc"]
    in_maps = _prep_inputs(np.asarray(x), np.asarray(p_q), np.asarray(p_k),
                           np.asarray(p_v), np.asarray(p_o))
    res = run_bass_kernel_spmd(nc, in_maps, core_ids=list(range(8)))
    parts = [r["out_part"].astype(np.float32) for r in res.results]
    out = np.stack([parts[0] + parts[1] + parts[2] + parts[3],
                    parts[4] + parts[5] + parts[6] + parts[7]])
    return out.astype(np.float32)



# revision 15
# speedup vs baseline: 1.1639x; 1.1639x over previous
"""Multi-head self-attention (B=2, S=2048, D=1024, H=16, causal) on 8 TRN2 NeuronCores.

Sharding: data parallel over batch (2) x tensor parallel over heads (4 groups of 4).
Core c handles batch c//4, heads 4*(c%4) .. 4*(c%4)+4; host sums 4 partials/batch.

Design (v5) - fp32r datapath:
- On this HW the PE fp32r path matches fp32 numerics (~1.4e-4 rms; the
  "12-mantissa-bit" caveat applies to CoreSim, not HW) and runs at bf16 speed
  when the moving dim is >= 256. So Q/K/V projections are single-pass fp32r
  (1/3 the matmuls of the old bf16 hi/lo scheme) and scores are a single
  fp32r matmul per 512-chunk (1/2 the old two-stream bf16 scheme).
- Q/K stored fp32r as [128, dc, seq]: head 2dc+0 on partitions 0:64, head
  2dc+1 on partitions 64:128 (offset-64 matmul operands verified on HW), so
  projection PSUM -> SBUF is one full-width engine copy, no cross-partition
  DMA.
- Scores accumulate in wide PSUM tiles (3 banks = 1536 cols) -> one DVE/Pool
  max-reduce + one Act exp per (qi,head) chunk instead of per-512 ops.
- P fp16 packed per-qi [128, 4*kend]; ONE dma transpose per head-pair
  -> pt [128, 4, kt, 128]; fp16 attn@V with ones-column denominators.
- Output path fp16: cat/acT/poT/out store all fp16.
- Causal mask applied with a bf16 ident x mask matmul into the fp32r score
  group (mixed-dtype PSUM group verified on HW).
"""
import sys
for _p in ("/opt/trn_rl_repo", "/root/.axon_site/_ro/trn_rl_repo"):
    if _p not in sys.path:
        sys.path.append(_p)

import math
from contextlib import ExitStack

import numpy as np
import ml_dtypes

import concourse.bass as bass
import concourse.bacc as bacc
import concourse.tile as tile
import concourse.mybir as mybir
from concourse.bass_utils import run_bass_kernel_spmd

BF16 = mybir.dt.bfloat16
FP16 = mybir.dt.float16
F32 = mybir.dt.float32
F32R = mybir.dt.float32r
SEQ = 2048
DM = 1024
DL = 256          # local head dims (4 heads x 64)
DH = 64
MC = 8            # 128-row chunks of the model dim
NQT = SEQ // 128  # 16 q tiles
NEG = -3.0e30     # causal mask addend
VW = 66           # v columns per head: 64 + ones col + pad
SCH = 1024        # score-chunk width (2 PSUM banks)

_CACHE = {}


def build_nc():
    nc = bacc.Bacc("TRN2", debug=False, num_devices=8)

    x_d = nc.dram_tensor("x", [128, MC, SEQ], F32R, kind="ExternalInput")
    wq_d = nc.dram_tensor("wq", [128, MC, DL], F32R, kind="ExternalInput")
    wk_d = nc.dram_tensor("wk", [128, MC, DL], F32R, kind="ExternalInput")
    wv_d = nc.dram_tensor("wv", [128, MC, DL], F32R, kind="ExternalInput")
    poT_d = nc.dram_tensor("poT", [128, 2, DM], FP16, kind="ExternalInput")
    mask_d = nc.dram_tensor("mask", [128, 128], BF16, kind="ExternalInput")
    ident_d = nc.dram_tensor("ident", [128, 128], BF16, kind="ExternalInput")
    out_d = nc.dram_tensor("out_part", [SEQ, DM], FP16, kind="ExternalOutput")

    X = mybir.AxisListType.X
    EXPF = mybir.ActivationFunctionType.Exp
    CPY = mybir.ActivationFunctionType.Copy

    with tile.TileContext(nc) as tc, ExitStack() as ctx:
        cst = ctx.enter_context(tc.tile_pool(name="cst", bufs=1))
        xp = ctx.enter_context(tc.tile_pool(name="xp", bufs=2))
        pp = ctx.enter_context(tc.tile_pool(name="pp", bufs=2))
        ptp = ctx.enter_context(tc.tile_pool(name="ptp", bufs=2))
        catp = ctx.enter_context(tc.tile_pool(name="catp", bufs=2))
        acTp = ctx.enter_context(tc.tile_pool(name="acTp", bufs=2))
        outp = ctx.enter_context(tc.tile_pool(name="outp", bufs=2))
        small = ctx.enter_context(tc.tile_pool(name="small", bufs=16))
        sp = ctx.enter_context(tc.tile_pool(name="sp", bufs=3, space="PSUM"))
        mp = ctx.enter_context(tc.tile_pool(name="mp", bufs=2, space="PSUM"))

        # ---- persistent SBUF ----
        mask_sb = cst.tile([128, 128], BF16, tag="mask")
        ident_sb = cst.tile([128, 128], BF16, tag="ident")
        poT_sb = cst.tile([128, 2, DM], FP16, tag="poT")
        wsb = {}
        for nm_, dr_ in (("wq", wq_d), ("wk", wk_d), ("wv", wv_d)):
            wsb[nm_] = cst.tile([128, MC, DL], F32R, tag=nm_, name=nm_)
        Qt = cst.tile([128, 2, SEQ], F32R, tag="Qt")
        Kt = cst.tile([128, 2, SEQ], F32R, tag="Kt")
        v_sb = cst.tile([128, NQT, 4, VW], FP16, tag="v")

        # input DMAs: one per tensor (batched descriptors)
        nc.sync.dma_start(out=mask_sb, in_=mask_d[:, :])
        nc.sync.dma_start(out=ident_sb, in_=ident_d[:, :])
        nc.sync.dma_start(out=wsb["wq"][:, :, :], in_=wq_d[:, :, :])
        nc.sync.dma_start(out=wsb["wk"][:, :, :], in_=wk_d[:, :, :])
        nc.sync.dma_start(out=wsb["wv"][:, :, :], in_=wv_d[:, :, :])
        nc.scalar.dma_start(out=poT_sb[:, :, :], in_=poT_d[:, :, :])
        nc.gpsimd.memset(v_sb[:, :, :, 64:VW], 0.0)
        nc.gpsimd.memset(v_sb[:, :, :, 64:65], 1.0)

        xs_tiles = {}

        def load_span(s, q):
            t = xp.tile([128, MC, 512], F32R, tag="xs", name=f"xs{s}")
            q.dma_start(out=t[:, :, :], in_=x_d[:, :, 512 * s:512 * (s + 1)])
            xs_tiles[s] = t

        load_span(0, nc.sync)
        load_span(1, nc.scalar)

        # ---- projections ----
        def emit_chain(s, t_, dc):
            """Q or K projection for 512-seq span s, head-pair dc."""
            xs = xs_tiles[s]
            span = slice(512 * s, 512 * (s + 1))
            dst = Qt if t_ == "wq" else Kt
            pr = mp.tile([128, 512], F32, tag="m", name="pr")
            for m in range(MC):
                nc.tensor.matmul(pr, wsb[t_][:, m, 128 * dc:128 * (dc + 1)],
                                 xs[:, m, :], start=(m == 0), stop=(m == MC - 1))
            nc.scalar.copy(out=dst[:, dc, span], in_=pr)

        def emit_v(qi):
            s = qi // 4
            xs = xs_tiles[s]
            qoff = 128 * (qi % 4)
            vps = mp.tile([128, 512], F32, tag="m", name="vps")
            for m in range(MC):
                nc.tensor.matmul(vps[:, 0:DL], xs[:, m, qoff:qoff + 128],
                                 wsb["wv"][:, m, :], start=(m == 0), stop=(m == MC - 1))
            nc.gpsimd.tensor_copy(out=v_sb[:, qi, :, 0:64],
                                  in_=vps[:, 0:DL].rearrange("p (h d) -> p h d", h=4))

        # chain schedule: span s fully emitted across listed qi slots
        # (span 0 is handled specially inside qi=0 for correct ordering)
        CHAINS = {
            1: [("wq", 1, 0), ("wk", 1, 0)],
            2: [("wq", 1, 1), ("wk", 1, 1)],
            4: [("wq", 2, 0), ("wk", 2, 0)],
            5: [("wq", 2, 1), ("wk", 2, 1)],
            8: [("wq", 3, 0), ("wk", 3, 0)],
            9: [("wq", 3, 1), ("wk", 3, 1)],
        }
        VSLOT = {0: [1, 2, 3], 2: [4, 5, 6, 7], 5: [8, 9, 10, 11],
                 9: [12, 13, 14, 15]}
        XLOAD = {3: 2, 7: 3}

        # ---- attention ----
        state = {}

        def emit_scores(qi, hl):
            """S = q_hl . K[:kend] (+ causal mask on diag block), max, exp->pp."""
            kend = 128 * (qi + 1)
            dc, e = hl // 2, hl % 2
            prow = slice(64 * e, 64 * (e + 1))
            qspan = slice(128 * qi, 128 * (qi + 1))
            pb = state[("pp", qi)]
            chunks = [(0, min(SCH, kend))]
            if kend > SCH:
                chunks.append((SCH, kend - SCH))
            nms = []
            tiles = []
            for base, cw in chunks:
                S = sp.tile([128, SCH], F32, tag="s", name="S")
                tiles.append((S, base, cw))
                for cb in range(0, cw, 512):
                    w_ = min(512, cw - cb)
                    gb = base + cb
                    has_diag = gb <= 128 * qi < gb + w_
                    nc.tensor.matmul(S[:, cb:cb + w_], Qt[prow, dc, qspan],
                                     Kt[prow, dc, gb:gb + w_],
                                     start=True, stop=not has_diag,
                                     skip_group_check=True)
                    if has_diag:
                        off = 128 * qi - gb
                        nc.tensor.matmul(S[:, cb + off:cb + off + 128], ident_sb,
                                         mask_sb, start=False, stop=True,
                                         skip_group_check=True)
                nmc = small.tile([128, 1], F32, tag="nm", name="nmc")
                nc.vector.tensor_reduce(out=nmc, in_=S[:, :cw], axis=X,
                                        op=mybir.AluOpType.max, negate=True)
                nms.append(nmc)
            if len(nms) > 1:
                nm = small.tile([128, 1], F32, tag="nm", name="nm2")
                nc.gpsimd.tensor_tensor(out=nm, in0=nms[0], in1=nms[1],
                                        op=mybir.AluOpType.min)
            else:
                nm = nms[0]
            for S, base, cw in tiles:
                nc.scalar.activation(out=pb[:, hl * kend + base:hl * kend + base + cw],
                                     in_=S[:, :cw], func=EXPF, bias=nm, scale=1.0)

        def emit_transpose(qi, half):
            kend = 128 * (qi + 1)
            nkt = qi + 1
            pb = state[("pp", qi)]
            pt = state[("pt", qi)]
            nc.sync.dma_start_transpose(
                out=pt[:, 2 * half:2 * half + 2, :nkt, :].rearrange(
                    "p h k c -> p h (k c)"),
                in_=pb[:, 2 * half * kend:(2 * half + 2) * kend])

        def back_head(qj, hl):
            """attn@V + normalization for head hl of query tile qj."""
            nkt = qj + 1
            pt = state[("pt", qj)]
            cat = state[("cat", qj)]
            av = mp.tile([128, 512], F32, tag="m", name="av")
            for kt in range(nkt):
                nc.tensor.matmul(av[:, 0:VW], pt[:, hl, kt, :],
                                 v_sb[:, kt, hl, :],
                                 start=(kt == 0), stop=(kt == nkt - 1))
            inv = small.tile([128, 1], F32, tag="inv", name="inv")
            nc.vector.reciprocal(out=inv, in_=av[:, 64:65])
            eng = nc.gpsimd if hl % 2 == 0 else nc.scalar
            if hl % 2 == 0:
                nc.gpsimd.tensor_scalar_mul(out=cat[:, 64 * hl:64 * (hl + 1)],
                                            in0=av[:, 0:64], scalar1=inv)
            else:
                nc.scalar.activation(out=cat[:, 64 * hl:64 * (hl + 1)],
                                     in_=av[:, 0:64], func=CPY,
                                     scale=inv, bias=0.0)

        def back_tail(qj):
            cat = state.pop(("cat", qj))
            acT = acTp.tile([128, 2, 128], FP16, tag="acT", name="acT")
            nc.sync.dma_start_transpose(out=acT, in_=cat)
            osb = outp.tile([128, DM], FP16, tag="osb", name="osb")
            for nci in range(2):
                ops = mp.tile([128, 512], F32, tag="m", name="ops")
                for mlc in range(2):
                    nc.tensor.matmul(ops, acT[:, mlc, :],
                                     poT_sb[:, mlc, 512 * nci:512 * (nci + 1)],
                                     start=(mlc == 0), stop=(mlc == 1))
                nc.gpsimd.tensor_copy(out=osb[:, 512 * nci:512 * (nci + 1)],
                                      in_=ops)
            nc.sync.dma_start(out=out_d[128 * qj:128 * (qj + 1), :], in_=osb)
            state.pop(("pt", qj))
            state.pop(("pp", qj))

        for qi in range(NQT):
            state[("pp", qi)] = pp.tile([128, 4 * SEQ], FP16, tag="p",
                                        name=f"p{qi}")
            state[("pt", qi)] = ptp.tile([128, 4, NQT, 128], FP16, tag="pt",
                                         name=f"pt{qi}")
            state[("cat", qi)] = catp.tile([128, DL], FP16, tag="cat",
                                           name="cat")
            chains = list(CHAINS.get(qi, []))
            if qi == 0:
                # span-0 chains must precede the first scores
                emit_chain(0, "wq", 0)
                emit_chain(0, "wk", 0)
                chains = [("wq", 0, 1), ("wk", 0, 1)]
                emit_v(0)
            for hl in range(4):
                if qi == 0 and hl == 2:
                    # dc=1 heads need the dc=1 chains first
                    while chains:
                        t_, s_, dc_ = chains.pop(0)
                        emit_chain(s_, t_, dc_)
                emit_scores(qi, hl)
                if hl == 1:
                    emit_transpose(qi, 0)
                if hl == 3:
                    emit_transpose(qi, 1)
                if chains:
                    t_, s_, dc_ = chains.pop(0)
                    emit_chain(s_, t_, dc_)
                if qi > 0:
                    back_head(qi - 1, hl)
            for v_ in VSLOT.get(qi, []):
                emit_v(v_)
            if qi in XLOAD:
                load_span(XLOAD[qi], nc.sync)
            if qi > 0:
                back_tail(qi - 1)
        for hl in range(4):
            back_head(NQT - 1, hl)
        back_tail(NQT - 1)

    nc.compile()
    return nc


def _prep_inputs(x, p_q, p_k, p_v, p_o):
    """Build the 8 per-core input maps."""
    per_batch = []
    for b in range(2):
        xT = np.ascontiguousarray(x[b].T).astype(np.float32)      # [1024, 2048]
        xr = np.ascontiguousarray(
            xT.reshape(MC, 128, SEQ).transpose(1, 0, 2))          # [128, MC, SEQ]
        per_batch.append(xr)

    mask = np.zeros((128, 128), np.float32)
    iu = np.triu_indices(128, 1)
    mask[iu] = NEG
    mask = mask.astype(ml_dtypes.bfloat16)
    ident = np.eye(128, dtype=ml_dtypes.bfloat16)

    def wprep(w):
        # [256 out, 1024 in] -> lhsT layout [128 part(m), MC, 256]
        wT = np.ascontiguousarray(w.T).astype(np.float32)         # [1024, 256]
        return np.ascontiguousarray(wT.reshape(MC, 128, DL).transpose(1, 0, 2))

    per_group = []
    for g in range(4):
        rows = slice(DL * g, DL * (g + 1))
        poT = np.ascontiguousarray(p_o[:, rows].T).astype(np.float16)  # [256,1024]
        per_group.append(dict(
            wq=wprep(p_q[rows] / math.sqrt(DH)),
            wk=wprep(p_k[rows]),
            wv=wprep(p_v[rows]),
            poT=np.ascontiguousarray(poT.reshape(2, 128, DM).transpose(1, 0, 2)),
        ))

    in_maps = []
    for c in range(8):
        b, g = c // 4, c % 4
        m = dict(per_group[g])
        m["x"] = per_batch[b]
        m["mask"] = mask
        m["ident"] = ident
        in_maps.append(m)
    return in_maps


def kernel(x, p_q, p_k, p_v, p_o):
    if "nc" not in _CACHE:
        _CACHE["nc"] = build_nc()
    nc = _CACHE["nc"]
    in_maps = _prep_inputs(np.asarray(x), np.asarray(p_q), np.asarray(p_k),
                           np.asarray(p_v), np.asarray(p_o))
    res = run_bass_kernel_spmd(nc, in_maps, core_ids=list(range(8)))
    parts = [r["out_part"].astype(np.float32) for r in res.results]
    out = np.stack([parts[0] + parts[1] + parts[2] + parts[3],
                    parts[4] + parts[5] + parts[6] + parts[7]])
    return out.astype(np.float32)


# revision 40
# speedup vs baseline: 1.3165x; 1.1311x over previous
"""Multi-head self-attention (B=2, S=2048, D=1024, H=16, causal) on 8 TRN2 NeuronCores.

Sharding: data parallel over batch (2) x tensor parallel over heads (4 groups of 4).
Core c handles batch c//4, heads 4*(c%4) .. 4*(c%4)+4; host sums 4 partials/batch.

Design (v5) - fp32r datapath:
- On this HW the PE fp32r path matches fp32 numerics (~1.4e-4 rms; the
  "12-mantissa-bit" caveat applies to CoreSim, not HW) and runs at bf16 speed
  when the moving dim is >= 256. So Q/K/V projections are single-pass fp32r
  (1/3 the matmuls of the old bf16 hi/lo scheme) and scores are a single
  fp32r matmul per 512-chunk (1/2 the old two-stream bf16 scheme).
- Q/K stored fp32r as [128, dc, seq]: head 2dc+0 on partitions 0:64, head
  2dc+1 on partitions 64:128 (offset-64 matmul operands verified on HW), so
  projection PSUM -> SBUF is one full-width engine copy, no cross-partition
  DMA.
- Scores accumulate in wide PSUM tiles (3 banks = 1536 cols) -> one DVE/Pool
  max-reduce + one Act exp per (qi,head) chunk instead of per-512 ops.
- P fp16 packed per-qi [128, 4*kend]; ONE dma transpose per head-pair
  -> pt [128, 4, kt, 128]; fp16 attn@V with ones-column denominators.
- Output path fp16: cat/acT/poT/out store all fp16.
- Causal mask applied with a bf16 ident x mask matmul into the fp32r score
  group (mixed-dtype PSUM group verified on HW).
"""
import sys
for _p in ("/opt/trn_rl_repo", "/root/.axon_site/_ro/trn_rl_repo"):
    if _p not in sys.path:
        sys.path.append(_p)

import math
from contextlib import ExitStack

import numpy as np
import ml_dtypes

import concourse.bass as bass
import concourse.bacc as bacc
import concourse.tile as tile
import concourse.mybir as mybir
from concourse.bass_utils import run_bass_kernel_spmd

BF16 = mybir.dt.bfloat16
FP16 = mybir.dt.float16
F32 = mybir.dt.float32
F32R = mybir.dt.float32r
SEQ = 2048
DM = 1024
DL = 256          # local head dims (4 heads x 64)
DH = 64
MC = 8            # 128-row chunks of the model dim
NQT = SEQ // 128  # 16 q tiles
NEG = -3.0e30     # causal mask addend
VW = 66           # v columns per head: 64 + ones col + pad
SCH = 512         # score-chunk width (1 PSUM bank)

_CACHE = {}


def build_nc():
    nc = bacc.Bacc("TRN2", debug=False, num_devices=8)

    x_d = nc.dram_tensor("x", [128, MC, SEQ], F32R, kind="ExternalInput")
    wq_d = nc.dram_tensor("wq", [128, MC, DL], F32R, kind="ExternalInput")
    wk_d = nc.dram_tensor("wk", [128, MC, DL], F32R, kind="ExternalInput")
    wv_d = nc.dram_tensor("wv", [128, MC, DL], F32R, kind="ExternalInput")
    poT_d = nc.dram_tensor("poT", [128, 2, DM], FP16, kind="ExternalInput")
    mask_d = nc.dram_tensor("mask", [128, 128], BF16, kind="ExternalInput")
    ident_d = nc.dram_tensor("ident", [128, 128], BF16, kind="ExternalInput")
    out_d = nc.dram_tensor("out_part", [SEQ, DM], FP16, kind="ExternalOutput")

    X = mybir.AxisListType.X
    EXPF = mybir.ActivationFunctionType.Exp
    CPY = mybir.ActivationFunctionType.Copy

    with tile.TileContext(nc) as tc, ExitStack() as ctx:
        cst = ctx.enter_context(tc.tile_pool(name="cst", bufs=1))
        xp = ctx.enter_context(tc.tile_pool(name="xp", bufs=2))
        pp = ctx.enter_context(tc.tile_pool(name="pp", bufs=3))
        ptp = ctx.enter_context(tc.tile_pool(name="ptp", bufs=3))
        catp = ctx.enter_context(tc.tile_pool(name="catp", bufs=3))
        acTp = ctx.enter_context(tc.tile_pool(name="acTp", bufs=3))
        outp = ctx.enter_context(tc.tile_pool(name="outp", bufs=3))
        small = ctx.enter_context(tc.tile_pool(name="small", bufs=16))
        sp = ctx.enter_context(tc.tile_pool(name="sp", bufs=6, space="PSUM"))
        mp = ctx.enter_context(tc.tile_pool(name="mp", bufs=2, space="PSUM"))

        # ---- persistent SBUF ----
        mask_sb = cst.tile([128, 128], BF16, tag="mask")
        ident_sb = cst.tile([128, 128], BF16, tag="ident")
        poT_sb = cst.tile([128, 2, DM], FP16, tag="poT")
        wsb = {}
        for nm_, dr_ in (("wq", wq_d), ("wk", wk_d), ("wv", wv_d)):
            wsb[nm_] = cst.tile([128, MC, DL], F32R, tag=nm_, name=nm_)
        Qt = cst.tile([128, 2, SEQ], F32R, tag="Qt")
        Kt = cst.tile([128, 2, SEQ], F32R, tag="Kt")
        v_sb = cst.tile([128, NQT, 4, VW], FP16, tag="v")

        xs_tiles = {}

        def load_span(s, q, halves=False):
            t = xp.tile([128, MC, 512], F32R, tag="xs", name=f"xs{s}")
            span = slice(512 * s, 512 * (s + 1))
            if halves:
                q.dma_start(out=t[:, 0:4, :], in_=x_d[:, 0:4, span])
            else:
                q.dma_start(out=t[:, :, :], in_=x_d[:, :, span])
            xs_tiles[s] = t
            return t

        # input DMAs in strict order on one queue so the DMA device serves
        # them in the order the compute consumes them
        nc.sync.dma_start(out=mask_sb, in_=mask_d[:, :])
        nc.sync.dma_start(out=ident_sb, in_=ident_d[:, :])
        t0 = load_span(0, nc.sync, halves=True)
        nc.sync.dma_start(out=wsb["wq"][:, 0:4, :], in_=wq_d[:, 0:4, :])
        nc.sync.dma_start(out=t0[:, 4:8, :], in_=x_d[:, 4:8, 0:512])
        nc.sync.dma_start(out=wsb["wq"][:, 4:8, :], in_=wq_d[:, 4:8, :])
        nc.sync.dma_start(out=wsb["wk"][:, 0:4, :], in_=wk_d[:, 0:4, :])
        nc.sync.dma_start(out=wsb["wk"][:, 4:8, :], in_=wk_d[:, 4:8, :])
        nc.sync.dma_start(out=wsb["wv"][:, :, :], in_=wv_d[:, :, :])
        t1 = load_span(1, nc.sync, halves=True)
        nc.sync.dma_start(out=t1[:, 4:8, :], in_=x_d[:, 4:8, 512:1024])
        nc.sync.dma_start(out=poT_sb[:, :, :], in_=poT_d[:, :, :])
        nc.gpsimd.memset(v_sb[:, :, :, 64:VW], 0.0)
        nc.gpsimd.memset(v_sb[:, :, :, 64:65], 1.0)

        # ---- projections ----
        def emit_chain(s, t_, dc):
            """Q or K projection for 512-seq span s, head-pair dc."""
            xs = xs_tiles[s]
            span = slice(512 * s, 512 * (s + 1))
            dst = Qt if t_ == "wq" else Kt
            pr = mp.tile([128, 512], F32, tag="m", name="pr")
            for m in range(MC):
                nc.tensor.matmul(pr, wsb[t_][:, m, 128 * dc:128 * (dc + 1)],
                                 xs[:, m, :], start=(m == 0), stop=(m == MC - 1))
            nc.scalar.copy(out=dst[:, dc, span], in_=pr)

        def emit_v(qi):
            s = qi // 4
            xs = xs_tiles[s]
            qoff = 128 * (qi % 4)
            vps = mp.tile([128, 512], F32, tag="m", name="vps")
            for m in range(MC):
                nc.tensor.matmul(vps[:, 0:DL], xs[:, m, qoff:qoff + 128],
                                 wsb["wv"][:, m, :], start=(m == 0), stop=(m == MC - 1))
            nc.vector.tensor_copy(out=v_sb[:, qi, :, 0:64],
                                   in_=vps[:, 0:DL].rearrange("p (h d) -> p h d", h=4))

        # chain schedule: span s fully emitted across listed qi slots
        # (span 0 is handled specially inside qi=0 for correct ordering)
        CHAINS = {
            1: [("wq", 1, 0), ("wk", 1, 0)],
            2: [("wq", 1, 1), ("wk", 1, 1)],
            3: [("wq", 2, 0), ("wk", 2, 0)],
            4: [("wq", 2, 1), ("wk", 2, 1)],
            5: [("wq", 3, 0), ("wk", 3, 0)],
            6: [("wq", 3, 1), ("wk", 3, 1)],
        }
        VSLOT = {1: [1, 2, 3], 2: [4, 5], 3: [6, 7], 5: [8, 9, 10, 11],
                 6: [12, 13], 7: [14, 15]}
        XLOAD = {2: 2, 4: 3}

        # ---- attention ----
        state = {}

        def emit_scores(qi, hl):
            """S = q_hl . K[:kend] (+ causal mask on diag block), max, exp->pp."""
            kend = 128 * (qi + 1)
            dc, e = hl // 2, hl % 2
            prow = slice(64 * e, 64 * (e + 1))
            qspan = slice(128 * qi, 128 * (qi + 1))
            pb = state[("pp", qi)]
            chunks = [(0, min(SCH, kend))]
            b = SCH
            while b < kend:
                cw = min(512, kend - b)
                chunks.append((b, cw))
                b += cw
            nms = []
            tiles = []
            for base, cw in chunks:
                S = sp.tile([128, SCH], F32, tag="s", name="S")
                tiles.append((S, base, cw))
                for cb in range(0, cw, 512):
                    w_ = min(512, cw - cb)
                    gb = base + cb
                    has_diag = gb <= 128 * qi < gb + w_
                    nc.tensor.matmul(S[:, cb:cb + w_], Qt[prow, dc, qspan],
                                     Kt[prow, dc, gb:gb + w_],
                                     start=True, stop=not has_diag,
                                     skip_group_check=True)
                    if has_diag:
                        off = 128 * qi - gb
                        nc.tensor.matmul(S[:, cb + off:cb + off + 128], ident_sb,
                                         mask_sb, start=False, stop=True,
                                         skip_group_check=True)
                nmc = small.tile([128, 1], F32, tag="nm", name="nmc")
                nc.vector.tensor_reduce(out=nmc, in_=S[:, :cw], axis=X,
                                        op=mybir.AluOpType.max, negate=True)
                nms.append(nmc)
            while len(nms) > 1:
                nm2 = small.tile([128, 1], F32, tag="nm", name="nm2")
                nc.vector.tensor_tensor(out=nm2, in0=nms[0], in1=nms[1],
                                         op=mybir.AluOpType.min)
                nms = [nm2] + nms[2:]
            nm = nms[0]
            for S, base, cw in tiles:
                nc.scalar.activation(
                    out=pb[:, base // 128:(base + cw) // 128, hl, :],
                    in_=S[:, :cw], func=EXPF, bias=nm, scale=1.0)

        def emit_transpose(qi, half):
            nkt = qi + 1
            k0 = (nkt + 1) // 2
            ka, kb = (0, k0) if half == 0 else (k0, nkt)
            if ka == kb:
                return
            pb = state[("pp", qi)]
            pt = state[("pt", qi)]
            nc.sync.dma_start_transpose(
                out=pt[:, ka:kb, :, :].rearrange("p k h c -> p (k h) c"),
                in_=pb[:, ka:kb, :, :].rearrange("p k h c -> p (k h c)"))

        def back_head(qj, hl):
            """attn@V for head hl of query tile qj; all 4 heads share one
            PSUM tile (independent accumulation groups in one bank)."""
            nkt = qj + 1
            pt = state[("pt", qj)]
            if hl == 0:
                state[("av", qj)] = mp.tile([128, 4, VW], F32, tag="m",
                                            name="av")
            av = state[("av", qj)]
            for kt in range(nkt):
                nc.tensor.matmul(av[:, hl, :], pt[:, kt, hl, :],
                                 v_sb[:, kt, hl, :],
                                 start=(kt == 0), stop=(kt == nkt - 1))

        def back_scale(qj):
            """normalize attn-out and launch the cat transpose."""
            cat = state.pop(("cat", qj))
            av = state.pop(("av", qj))
            inv = small.tile([128, 4], F32, tag="inv", name="inv")
            nc.vector.reciprocal(out=inv, in_=av[:, :, 64])
            for hl in range(4):
                nc.vector.tensor_scalar_mul(
                    out=cat[:, 64 * hl:64 * (hl + 1)],
                    in0=av[:, hl, 0:64], scalar1=inv[:, hl:hl + 1])
            acT = acTp.tile([128, 2, 128], FP16, tag="acT", name="acT")
            nc.sync.dma_start_transpose(out=acT, in_=cat)
            state[("acT", qj)] = acT
            state.pop(("pt", qj))
            state.pop(("pp", qj))

        def back_out(qj):
            acT = state.pop(("acT", qj))
            osb = outp.tile([128, DM], FP16, tag="osb", name="osb")
            for nci in range(2):
                ops = mp.tile([128, 512], F32, tag="m", name="ops")
                for mlc in range(2):
                    nc.tensor.matmul(ops, acT[:, mlc, :],
                                     poT_sb[:, mlc, 512 * nci:512 * (nci + 1)],
                                     start=(mlc == 0), stop=(mlc == 1))
                nc.scalar.copy(out=osb[:, 512 * nci:512 * (nci + 1)], in_=ops)
            state[("osb", qj)] = osb

        def back_store(qj):
            osb = state.pop(("osb", qj))
            nc.sync.dma_start(out=out_d[128 * qj:128 * (qj + 1), :], in_=osb)

        for qi in range(NQT):
            state[("pp", qi)] = pp.tile([128, NQT, 4, 128], FP16, tag="p",
                                        name=f"p{qi}")
            state[("pt", qi)] = ptp.tile([128, NQT, 4, 128], FP16, tag="pt",
                                         name=f"pt{qi}")
            state[("cat", qi)] = catp.tile([128, DL], FP16, tag="cat",
                                           name="cat")
            chains = list(CHAINS.get(qi, []))
            if qi > 4:
                back_store(qi - 5)
            if qi == 0:
                # span-0 chains must precede the first scores
                emit_chain(0, "wq", 0)
                emit_chain(0, "wk", 0)
                chains = [("wq", 0, 1), ("wk", 0, 1)]
                emit_v(0)
            for hl in range(4):
                if qi == 0 and hl == 2:
                    # dc=1 heads need the dc=1 chains first
                    while chains:
                        t_, s_, dc_ = chains.pop(0)
                        emit_chain(s_, t_, dc_)
                emit_scores(qi, hl)
                if hl == 1:
                    emit_transpose(qi, 0)
                if hl == 3:
                    emit_transpose(qi, 1)
                if chains:
                    t_, s_, dc_ = chains.pop(0)
                    emit_chain(s_, t_, dc_)
                if qi > 1:
                    back_head(qi - 2, hl)
                if hl == 2 and qi > 3:
                    back_out(qi - 4)
            for v_ in VSLOT.get(qi, []):
                emit_v(v_)
            if qi in XLOAD:
                load_span(XLOAD[qi], nc.sync)
            if qi > 1:
                back_scale(qi - 2)
        for qj in (NQT - 2, NQT - 1):
            for hl in range(4):
                back_head(qj, hl)
        for qj in (NQT - 2, NQT - 1):
            back_scale(qj)
        for qj in (NQT - 4, NQT - 3, NQT - 2, NQT - 1):
            back_out(qj)
        for qj in (NQT - 5, NQT - 4, NQT - 3, NQT - 2, NQT - 1):
            back_store(qj)

    nc.compile()
    return nc


def _prep_inputs(x, p_q, p_k, p_v, p_o):
    """Build the 8 per-core input maps."""
    per_batch = []
    for b in range(2):
        xT = np.ascontiguousarray(x[b].T).astype(np.float32)      # [1024, 2048]
        xr = np.ascontiguousarray(
            xT.reshape(MC, 128, SEQ).transpose(1, 0, 2))          # [128, MC, SEQ]
        per_batch.append(xr)

    mask = np.zeros((128, 128), np.float32)
    iu = np.triu_indices(128, 1)
    mask[iu] = NEG
    mask = mask.astype(ml_dtypes.bfloat16)
    ident = np.eye(128, dtype=ml_dtypes.bfloat16)

    def wprep(w):
        # [256 out, 1024 in] -> lhsT layout [128 part(m), MC, 256]
        wT = np.ascontiguousarray(w.T).astype(np.float32)         # [1024, 256]
        return np.ascontiguousarray(wT.reshape(MC, 128, DL).transpose(1, 0, 2))

    per_group = []
    for g in range(4):
        rows = slice(DL * g, DL * (g + 1))
        poT = np.ascontiguousarray(p_o[:, rows].T).astype(np.float16)  # [256,1024]
        per_group.append(dict(
            wq=wprep(p_q[rows] / math.sqrt(DH)),
            wk=wprep(p_k[rows]),
            wv=wprep(p_v[rows]),
            poT=np.ascontiguousarray(poT.reshape(2, 128, DM).transpose(1, 0, 2)),
        ))

    in_maps = []
    for c in range(8):
        b, g = c // 4, c % 4
        m = dict(per_group[g])
        m["x"] = per_batch[b]
        m["mask"] = mask
        m["ident"] = ident
        in_maps.append(m)
    return in_maps


def kernel(x, p_q, p_k, p_v, p_o):
    if "nc" not in _CACHE:
        _CACHE["nc"] = build_nc()
    nc = _CACHE["nc"]
    in_maps = _prep_inputs(np.asarray(x), np.asarray(p_q), np.asarray(p_k),
                           np.asarray(p_v), np.asarray(p_o))
    res = run_bass_kernel_spmd(nc, in_maps, core_ids=list(range(8)))
    parts = [r["out_part"].astype(np.float32) for r in res.results]
    out = np.stack([parts[0] + parts[1] + parts[2] + parts[3],
                    parts[4] + parts[5] + parts[6] + parts[7]])
    return out.astype(np.float32)
